# revision 52
# baseline (speedup 1.0000x reference)
"""BiAttention kernel for Trainium2, 8 NeuronCores, data-parallel over batch.

Reference computation (per batch b):
    S[i,j] = w1.c_i + w2.q_j + w3.(c_i*q_j)
    A      = softmax(S, axis=j)
    U[i]   = sum_j A[i,j] q_j
    bmax_i = max_j A[i,j]
    h      = sum_i bmax_i c_i
    G      = concat([c, U, c*U, c*h], axis=-1)

Key restructuring (v2, S^T orientation):
  - softmax over j is invariant to the s_c[i] term -> w1 is dead.
  - s_q[j] = w2.q_j folds into the c operand:
        S[i,j] = sum_d (w3[d]*c[i,d] + w2[d]) * q[j,d]
    so cmod = w3*c + w2 (one fused tensor_scalar during the c^T psum
    evacuation) and no separate s_q machinery / K=1 matmuls exist.
  - S is computed TRANSPOSED (S^T[j,i], j on partitions): the exp
    evacuation then yields A^T directly, which is exactly the lhsT the
    U matmul needs -> the 8-per-row-tile PE transposes of A and their
    evacuation copies (8M elems of ACT/DVE traffic) all disappear.
  - A^T is stored bf16 (halves SBUF traffic + DVE max cost; U/Z use the
    same rounded A so normalization errors partially cancel).
  - Z_i falls out of the U matmul via a ones-column appended to q.
  - bmax_i: elementwise max tree over the 8 j-chunks of A^T (DVE+GpSimd),
    one small PE transpose, then a free-axis max reduce; b = bmax/Z.
  - h accumulates during the main loop via one [128,1]x[128,256] matmul
    per row-tile; h broadcast to 128 partitions via a K=1 ones matmul
    (no DRAM round-trip).
  - All DMA on HWDGE (nc.sync) so GpSimd is a free elementwise engine.
"""

import sys

if "/opt/trn_rl_repo" not in sys.path:
    sys.path.insert(0, "/opt/trn_rl_repo")

from contextlib import ExitStack

import numpy as np

import concourse.bass as bass
import concourse.bacc as bacc_mod
import concourse.tile as tile
from concourse import mybir
from concourse.bass_utils import run_bass_kernel_spmd
from concourse.masks import make_identity

B, Tc, Tq, D = 8, 4096, 1024, 256
P = 128
NT = Tc // P  # 32 context row-tiles
NG = NT // 2  # 16 groups of 2 row-tiles
JC = Tq // P  # 8 question chunks
KC = D // P  # 2 feature chunks
N_CORES = 8
F32 = mybir.dt.float32
R32 = mybir.dt.float32r
BF16 = mybir.dt.bfloat16
EXP = mybir.ActivationFunctionType.Exp
MAX = mybir.AluOpType.max
MULT = mybir.AluOpType.mult
ADD = mybir.AluOpType.add


def _build_program() -> bass.Bass:
    nc = bacc_mod.Bacc()
    c_dram = nc.declare_dram_parameter("context", [Tc, D], F32, isOutput=False)
    q_dram = nc.declare_dram_parameter("question", [Tq, D], F32, isOutput=False)
    w_dram = nc.declare_dram_parameter("w", [3 * D, 1], F32, isOutput=False)
    g_dram = nc.declare_dram_parameter("out", [Tc, 4 * D], F32, isOutput=True)

    with ExitStack() as ctx:
        tc = ctx.enter_context(tile.TileContext(nc))
        singles = ctx.enter_context(tc.tile_pool(name="singles", bufs=1))
        work = ctx.enter_context(tc.tile_pool(name="work", bufs=3))
        at_pool = ctx.enter_context(tc.tile_pool(name="atp", bufs=3))
        cm_pool = ctx.enter_context(tc.tile_pool(name="cmp", bufs=3))
        cr_pool = ctx.enter_context(tc.tile_pool(name="crp", bufs=4))
        stg_pool = ctx.enter_context(tc.tile_pool(name="stgp", bufs=3))
        ch_pool = ctx.enter_context(tc.tile_pool(name="chp", bufs=5))
        ps_st = ctx.enter_context(tc.tile_pool(name="ps_st", bufs=2, space="PSUM"))
        ps_tp = ctx.enter_context(tc.tile_pool(name="ps_tp", bufs=1, space="PSUM"))
        ps_u = ctx.enter_context(tc.tile_pool(name="ps_u", bufs=2, space="PSUM"))
        ps_h = ctx.enter_context(tc.tile_pool(name="ps_h", bufs=1, space="PSUM"))

        # ---------------- prep (once per batch) ----------------
        ident = singles.tile([P, P], F32)
        make_identity(nc, ident)
        ident_bf = singles.tile([P, P], BF16)
        nc.vector.tensor_copy(ident_bf, ident)

        # w: single DMA, [128, 6] with col c = w[c*128 + p]
        # cols 0,1 = w1 (dead), 2,3 = w2, 4,5 = w3
        w_sb = singles.tile([P, 6, 1], F32)
        nc.sync.dma_start(out=w_sb, in_=w_dram.rearrange("(c p) o -> p c o", p=P))

        # PE warmup while DMAs fly: ~3.4us of transposes flips HAM to 8/8
        for wu in range(6):
            tpw = ps_tp.tile([P, 512], F32, tag="tp")
            for k in range(4):
                nc.tensor.transpose(tpw[:, k * P : (k + 1) * P], ident, ident)

        # question prep (DMA'd in halves; emitted after group 0's c-transposes
        # so the PE FIFO isn't blocked waiting on the q DMA)
        # qT/cmodT are bf16: LDWEIGHTS for a bf16 stationary is ~2x faster
        # than fp32r (97 vs 186 ns), and it paces the S^T matmul stream.
        q_raw = singles.tile([P, JC, D], F32)
        q_aug = singles.tile([P, JC, D + 1], BF16)
        qT = singles.tile([P, KC, Tq], BF16)

        def emit_q_prep():
            for qh in range(2):
                nc.sync.dma_start(
                    out=q_raw[:, qh * 4 : (qh + 1) * 4, :],
                    in_=q_dram[qh * 4 * P : (qh + 1) * 4 * P, :].rearrange(
                        "(jc p) d -> p jc d", p=P
                    ),
                )
            # q^T via PE transposes; lhsT of the S^T matmul.
            # Batches go through the (2-buf) ps_u pool so evacs overlap.
            for jg in range(2):
                for kc in range(KC):
                    tp = ps_u.tile([P, 4, P], F32, tag="u")
                    for j4 in range(4):
                        jc = jg * 4 + j4
                        nc.tensor.transpose(
                            tp[:, j4, :],
                            q_raw[:, jc, kc * P : (kc + 1) * P],
                            ident,
                        )
                    nc.vector.tensor_copy(
                        qT[:, kc, jg * 512 : (jg + 1) * 512], tp
                    )
            # bf16 copy with a ones column (U matmul rhs; col D = 1 -> Z)
            nc.vector.memset(q_aug[:, :, D : D + 1], 1.0)
            nc.vector.tensor_copy(q_aug[:, :, 0:D], q_raw)

        # ones column (fp32r) for the h-broadcast K=1 matmul
        ones_col = singles.tile([1, P], R32)
        nc.vector.memset(ones_col.bitcast(F32), 1.0)

        c_all = singles.tile([P, NT, D], F32)
        b_all = singles.tile([P, NT], R32)

        # -------- software-pipelined main loop over groups of 2 row-tiles ----
        # stage lag: U/bmax/stores of group g emitted one iteration behind
        # S^T of group g; h-matmul three behind.
        state = {}  # per-group saved handles
        cstate = {}  # front_c -> front_s handles
        stg_state = {}  # 2-group merged store staging

        def load_c(t0):
            nc.sync.dma_start(
                out=c_all[:, t0 : t0 + 8, :],
                in_=c_dram[t0 * P : (t0 + 8) * P, :].rearrange(
                    "(t p) d -> p t d", p=P
                ),
            )

        def front_c(g):
            """c prefetch, c^T transposes, cmod evac."""
            t0 = 2 * g
            if t0 % 8 == 0 and 0 < t0 and t0 + 8 < NT:
                load_c(t0 + 8)
            # c^T for both tiles: psum [128, (t 2), (kc 2), 128]
            tp = ps_tp.tile([P, 2, KC, P], F32, tag="tp")
            for tt in range(2):
                for kc in range(KC):
                    nc.tensor.transpose(
                        tp[:, tt, kc, :],
                        c_all[:, t0 + tt, kc * P : (kc + 1) * P],
                        ident,
                    )
            # cmodT[d, i] = w3[d] * c^T[d, i] + w2[d]; one fused op per kc
            cmodT = cm_pool.tile([P, KC, 2, P], BF16, tag="cm")
            for kc in range(KC):
                nc.vector.tensor_scalar(
                    cmodT[:, kc],
                    tp[:, :, kc, :],
                    w_sb[:, 4 + kc, :],
                    w_sb[:, 2 + kc, :],
                    MULT,
                    ADD,
                )
            cstate[g] = cmodT

        def front_s(g):
            """S^T matmuls, exp -> A^T, c_r copy."""
            t0 = 2 * g
            if g == 0:
                load_c(8)
            cmodT = cstate.pop(g)
            # plain fp32r copy of c for the h matmul rhs (ACT, after exp in FIFO)
            c_r = cr_pool.tile([P, 2, D], R32, tag="cr")

            # S^T: ONE psum tile [128, (jc 8), 256] (4 banks); lhsT = qT
            # chunk, rhs = cmodT chunk (256 i-cols = both row-tiles).
            # kc is the OUTER loop so the first 8 matmuls only need
            # cmodT[0] (the kc=1 evac can still be in flight on DVE).
            # PSUM has_written bits make this safe: start=True only on the
            # first matmul to touch each 512-f32 bank (even jc); later
            # kc=0 matmuls hit cleared bits and overwrite, kc=1 accumulates.
            sts = [ps_st.tile([P, 4, 2 * P], F32, tag="st", name=f"st{g}_{h}")
                   for h in range(2)]
            for kc in range(KC):
                for half in range(2):
                    for j4 in range(4):
                        jc = half * 4 + j4
                        nc.tensor.matmul(
                            sts[half][:, j4, :],
                            lhsT=qT[:, kc, jc * P : (jc + 1) * P],
                            rhs=cmodT[:, kc],
                            start=(kc == 0 and j4 % 2 == 0),
                            stop=(kc == KC - 1),
                            skip_group_check=True,
                        )
            # A^T = exp(S^T), bf16, one ACT op per half (N=1024)
            AT = at_pool.tile([P, JC, 2 * P], BF16, tag="at")
            for half in range(2):
                nc.scalar.activation(
                    AT[:, half * 4 : (half + 1) * 4, :], sts[half], EXP
                )
            # c_r copy on DVE (pairs with U-evac moving to ACT)
            nc.vector.tensor_copy(c_r, c_all[:, t0 : t0 + 2, :])
            state[g] = (t0, AT, c_r)

        def emit_back(g):
            """Max tree + bmax first (so the mt transposes retire long
            before the next group's cT transposes reuse the tp bank),
            then U matmuls, evacs, cU, stores."""
            t0, AT, c_r = state.pop(g)
            # max tree over jc: 8 -> 1, three merged bf16 DVE ops
            n = work.tile([P, 4, 2 * P], BF16, tag="n")
            nc.vector.tensor_max(n, AT[:, 0:4, :], AT[:, 4:8, :])
            pp = work.tile([P, 2, 2 * P], BF16, tag="pp")
            nc.vector.tensor_max(pp, n[:, 0:2, :], n[:, 2:4, :])
            M = work.tile([P, 2 * P], BF16, tag="M")
            nc.vector.tensor_max(M, pp[:, 0, :], pp[:, 1, :])
            # bmax: transpose M -> [i, j0], one merged free-axis max
            mt = ps_tp.tile([P, 2, P], BF16, tag="tp")
            for tt in range(2):
                nc.tensor.transpose(
                    mt[:, tt, :], M[:, tt * P : (tt + 1) * P], ident_bf
                )
            bZ2 = work.tile([P, 2], F32, tag="bz")
            nc.vector.tensor_reduce(
                out=bZ2, in_=mt, axis=mybir.AxisListType.X, op=MAX
            )
            u_list = []
            for tt in range(2):
                u_ps = ps_u.tile([P, 512], F32, tag="u")
                for jc in range(JC):
                    nc.tensor.matmul(
                        u_ps[:, 0 : D + 1],
                        lhsT=AT[:, jc, tt * P : (tt + 1) * P],
                        rhs=q_aug[:, jc, :],
                        start=(jc == 0),
                        stop=(jc == JC - 1),
                    )
                u_list.append(u_ps)
            rZ2 = work.tile([P, 2], F32, tag="rz")
            # staging tile spans 2 groups (4 tiles) -> one 1MB store
            if g % 2 == 0:
                stg_state[0] = stg_pool.tile(
                    [P, 4, 2 * D], F32, tag="stg", name=f"stg{g}"
                )
            stage = stg_state[0]
            base = (g % 2) * 2
            for tt in range(2):
                t = t0 + tt
                u_ps = u_list[tt]
                nc.vector.reciprocal(rZ2[:, tt : tt + 1], u_ps[:, D : D + 1])
                # U = (U*Z) * (1/Z)  (ACT per-partition scale, psum -> staging)
                nc.scalar.mul(
                    stage[:, base + tt, 0:D], u_ps[:, 0:D], rZ2[:, tt : tt + 1]
                )
                # cU = U * c  on gpsimd (f32, SBUF only)
                nc.gpsimd.tensor_mul(
                    stage[:, base + tt, D : 2 * D],
                    stage[:, base + tt, 0:D],
                    c_all[:, t, :],
                )
            # b = bmax_raw / Z for both tiles in one tiny op
            nc.vector.tensor_mul(b_all[:, t0 : t0 + 2], bZ2, rZ2)
            if g % 2 == 1:
                # merged store of 4 tiles' [U | cU] (1 MB)
                nc.sync.dma_start(
                    out=g_dram[(t0 - 2) * P : (t0 + 2) * P, D : 3 * D].rearrange(
                        "(t p) d -> p t d", p=P
                    ),
                    in_=stage,
                )
            if t0 % 8 == 6:
                # merged c passthrough store of 8 tiles (1 MB)
                tc0 = t0 - 6
                nc.sync.dma_start(
                    out=g_dram[tc0 * P : (tc0 + 8) * P, 0:D].rearrange(
                        "(t p) d -> p t d", p=P
                    ),
                    in_=c_all[:, tc0 : tc0 + 8, :],
                )
            return c_r

        def emit_h(g, h_ps, c_r):
            t0 = 2 * g
            for tt in range(2):
                t = t0 + tt
                nc.tensor.matmul(
                    h_ps,
                    lhsT=b_all[:, t : t + 1],
                    rhs=c_r[:, tt, :],
                    start=(t == 0),
                    stop=(t == NT - 1),
                )

        h_ps = ps_h.tile([1, D], F32, tag="h")
        cr_state = {}
        load_c(0)
        for g in range(NG):
            front_c(g)
            if g == 0:
                emit_q_prep()
            front_s(g)
            if g >= 1:
                cr_state[g - 1] = emit_back(g - 1)
            if g >= 3:
                emit_h(g - 3, h_ps, cr_state.pop(g - 3))
        cr_state[NG - 1] = emit_back(NG - 1)
        for g in range(NG - 3, NG):
            emit_h(g, h_ps, cr_state.pop(g))

        # ---------------- epilogue: broadcast h, write c*h ----------------
        h_sb = work.tile([1, D], R32, tag="hsb")
        nc.vector.tensor_copy(h_sb, h_ps)
        hb_ps = ps_h.tile([P, D], F32, tag="h")
        nc.tensor.matmul(hb_ps, lhsT=ones_col, rhs=h_sb, start=True, stop=True)
        # duplicated broadcast so 2-tile-wide muls have a matching operand
        h_bcast = work.tile([P, 2, D], F32, tag="hbc")
        nc.vector.tensor_copy(h_bcast[:, 0, :], hb_ps)
        nc.vector.tensor_copy(h_bcast[:, 1, :], hb_ps)

        # 8 chunks of 4 row-tiles: 2 merged muls + one 512KB store each
        for cg in range(NT // 4):
            ch4 = ch_pool.tile([P, 4, D], F32, tag="ch4")
            for i in range(2):
                t = cg * 4 + i * 2
                eng = nc.vector if i % 2 == 0 else nc.gpsimd
                eng.tensor_mul(
                    ch4[:, i * 2 : i * 2 + 2, :], c_all[:, t : t + 2, :], h_bcast
                )
            eng = nc.gpsimd if cg % 2 == 0 else nc.sync
            eng.dma_start(
                out=g_dram[cg * 4 * P : (cg + 1) * 4 * P, 3 * D : 4 * D].rearrange(
                    "(t p) d -> p t d", p=P
                ),
                in_=ch4,
            )

    nc.finalize()
    return nc


_NC_CACHE = None


def kernel(context, question, w):
    global _NC_CACHE
    context = np.asarray(context, dtype=np.float32)
    question = np.asarray(question, dtype=np.float32)
    w = np.asarray(w, dtype=np.float32)

    if _NC_CACHE is None:
        _NC_CACHE = _build_program()
    nc = _NC_CACHE

    in_maps = [
        {"context": context[b], "question": question[b], "w": w} for b in range(B)
    ]
    res = run_bass_kernel_spmd(nc, in_maps, list(range(N_CORES)))
    return np.stack([res.results[b]["out"] for b in range(B)], axis=0)
